# revision 23
# baseline (speedup 1.0000x reference)
"""Trainium2 Bass kernel for nn_AvgModel (AvgResNet2 GNN, B=4 N=8192 D=128 NB=15).

Strategy: data-parallel over the node/batch columns across 8 cores (RC=4096
columns each, each core's columns inside a single batch). Exact global BN
statistics via one small AllReduce ([128,5] f32) per sublayer: per-core
partial sums of H (batch-slotted via a per-core one-hot) and of H^2.

Math per sub-layer (feature-major [128, RC], y = x + u, H := elu(y)+1):
  E = exp(y)  (ACT, per-batch bias u);  H = max(y+1, min(E, 1))  (one DVE STT)
  Trunk stored shifted: X' = x+1, so block end is X'_new = W'^T H + X' (+u)
  via an identity-matmul accumulate on PE and an ACT Copy writeback.
  BN folded into the matmul: W' = a1 (.) W_top; u collects beta/mu/gamma
  terms and the global-avg (ga) half contribution (W_bot^T gv + bias).
  Stats: sum(H) via DVE STT accum_out; sum(H^2) via GPSIMD STT accum_out.
Precision: H/W fp16, trunk X' fp16, PSUM f32, all stat accums f32.
"""
import numpy as np

import concourse.bass as bass
import concourse.tile as tile
from concourse import bacc, mybir
import concourse.bass_utils as bass_utils

F32 = mybir.dt.float32
F16 = mybir.dt.float16
AF = mybir.ActivationFunctionType
ALU = mybir.AluOpType
AXX = mybir.AxisListType.X

B, N, D, NB = 4, 8192, 128, 15
R = B * N                  # 32768
NCORES = 8
RC = R // NCORES           # 4096 columns per core
Q = 2048                   # column chunk
NCH = RC // Q              # 2
NSUB = 2 * NB              # 30
EPS = 1e-5

_CACHE = {}


def _build():
    nc = bacc.Bacc("TRN2", target_bir_lowering=False, debug=False,
                   num_devices=NCORES)

    def din(name, shape, dt):
        return nc.dram_tensor(name, list(shape), dt, kind="ExternalInput").ap()

    XFh = din("XFh", [6, RC], F16)          # this core's input columns
    W1h = din("W1h", [6, D], F16)
    WTh = din("WTh", [NSUB, D, D], F16)     # W[k][:128,:]
    WBh = din("WBh", [NSUB, D, D], F16)     # W[k][128:,:] (ga half)
    PK = din("PK", [D, NSUB * 8], F32)      # per layer: g1 be1 g2 be2 bias ...
    CB1 = din("CB1", [D, 1], F32)           # b1 + 1
    W2h = din("W2h", [D, 120], F16)
    Sh = din("Sh", [3, 120], F16)           # selector for +tile(inputs[...,-3:])
    CV = din("CV", [D, 4], F32)             # g2, be2, b2(pad)
    IDm = din("IDm", [D, D], F16)           # identity (residual accumulate)
    SEL = din("SEL", [D, 4], F32)           # one-hot of this core's batch
    OUT = nc.dram_tensor("OUT", [120, RC], F32, kind="ExternalOutput").ap()
    groups = [list(range(NCORES))]

    from contextlib import ExitStack
    with tile.TileContext(nc) as tc, ExitStack() as stk:
        sb = stk.enter_context(tc.tile_pool(name="sb", bufs=1))
        wp = stk.enter_context(tc.tile_pool(name="wp", bufs=2))
        ep = stk.enter_context(tc.tile_pool(name="ep", bufs=3))
        qp = stk.enter_context(tc.tile_pool(name="qp", bufs=2))
        tp = stk.enter_context(tc.tile_pool(name="tp", bufs=2))
        op_ = stk.enter_context(tc.tile_pool(name="op", bufs=2))
        ps = stk.enter_context(tc.tile_pool(name="ps", bufs=2, space="PSUM"))
        dr = stk.enter_context(tc.tile_pool(name="dr", bufs=2, space="DRAM"))

        Xp = sb.tile([D, RC], F16, tag="Xp")    # trunk, stored as x+1
        Ht = sb.tile([D, RC], F16, tag="Ht")
        pk_t = sb.tile([D, NSUB * 8], F32, tag="pk")
        nc.sync.dma_start(pk_t[:], PK[:])
        cb1_t = sb.tile([D, 1], F32, tag="cb1")
        nc.sync.dma_start(cb1_t[:], CB1[:])
        cv_t = sb.tile([D, 4], F32, tag="cv")
        nc.sync.dma_start(cv_t[:], CV[:])
        w2_t = sb.tile([D, 120], F16, tag="w2")
        nc.sync.dma_start(w2_t[:], W2h[:])
        s_t = sb.tile([3, 120], F16, tag="sel3")
        nc.sync.dma_start(s_t[:], Sh[:])
        w1_t = sb.tile([6, D], F16, tag="w1")
        nc.sync.dma_start(w1_t[:], W1h[:])
        id_t = sb.tile([D, D], F16, tag="id")
        nc.sync.dma_start(id_t[:], IDm[:])
        sel_t = sb.tile([D, 4], F32, tag="sel4")
        nc.sync.dma_start(sel_t[:], SEL[:])
        neg1 = sb.tile([D, 1], F32, tag="neg1")
        nc.vector.memset(neg1[:], -1.0)

        def square(c, qacc):
            """sum(H^2) for chunk c via ACT Square accum (Pool can't run STT;
            Square shares the natural_log_exp table set with Exp/Ln)."""
            cs = slice(c * Q, (c + 1) * Q)
            sq = qp.tile([D, Q], F16, tag="sq")
            nc.scalar.activation(sq[:], Ht[:, cs], AF.Square,
                                 accum_out=qacc[:, c:c + 1])

        def ew_A(c, hacc, qacc):
            """Elementwise from trunk Xp (= x+1): H = max(X', min(exp(X'-1),1))."""
            cs = slice(c * Q, (c + 1) * Q)
            E = ep.tile([D, Q], F16, tag="E")
            nc.scalar.activation(E[:], Xp[:, cs], AF.Exp, bias=neg1[:, 0:1],
                                 scale=1.0)
            nc.vector.scalar_tensor_tensor(
                Ht[:, cs], E[:], 1.0, Xp[:, cs], op0=ALU.min, op1=ALU.max,
                accum_out=hacc[:, c:c + 1])
            square(c, qacc)

        def ew_B(pt, uc, u1c, c, hacc, qacc):
            """Elementwise from PSUM x: H = max(x+u+1, min(exp(x+u),1))."""
            cs = slice(c * Q, (c + 1) * Q)
            E = ep.tile([D, Q], F16, tag="E")
            nc.scalar.activation(E[:], pt[:], AF.Exp, bias=uc, scale=1.0)
            Em = ep.tile([D, Q], F16, tag="E")
            nc.vector.tensor_scalar(Em[:], E[:], 1.0, None, ALU.min)
            nc.vector.scalar_tensor_tensor(
                Ht[:, cs], pt[:], u1c, Em[:], op0=ALU.add, op1=ALU.max,
                accum_out=hacc[:, c:c + 1])
            square(c, qacc)

        def collect(hacc, qacc):
            """AllReduce per-core partials -> [D,5]: batch H-sums | sum H^2."""
            st = tp.tile([D, 8], F32, tag="st")
            hsum = tp.tile([D, 1], F32, tag="hsum")
            nc.vector.tensor_reduce(hsum[:], hacc[:], axis=AXX, op=ALU.add)
            nc.vector.tensor_tensor(st[:, 0:4], hsum[:].broadcast_to((D, 4)),
                                    sel_t[:], op=ALU.mult)
            nc.vector.tensor_reduce(st[:, 4:5], qacc[:], axis=AXX, op=ALU.add)
            cin = dr.tile([D, 5], F32, tag="cin")
            nc.gpsimd.dma_start(cin[:], st[:, 0:5])
            cout = dr.tile([D, 5], F32, tag="cout")
            nc.gpsimd.collective_compute(
                "AllReduce", ALU.add, replica_groups=groups,
                ins=[cin.opt()], outs=[cout.opt()])
            sg = tp.tile([D, 5], F32, tag="sg")
            nc.sync.dma_start(sg[:], cout[:])
            return sg

        def chain(k, sg):
            """Global stats [D,5] -> (W'_k fp16, u_core [D,1], u_core+1)."""
            col = lambda j: pk_t[:, k * 8 + j:k * 8 + j + 1]
            g1, be1, g2, be2, bv = col(0), col(1), col(2), col(3), col(4)
            wt = wp.tile([D, D], F16, tag="wt")
            nc.sync.dma_start(wt[:], WTh[k, :, :])
            wb = wp.tile([D, D], F16, tag="wb")
            nc.sync.dma_start(wb[:], WBh[k, :, :])
            bs4 = sg[:, 0:4]
            qt = sg[:, 4:5]
            tot = tp.tile([D, 1], F32, tag="tot")
            nc.vector.tensor_reduce(tot[:], bs4, axis=AXX, op=ALU.add)
            muH = tp.tile([D, 1], F32, tag="muH")
            nc.vector.tensor_scalar(muH[:], tot[:], 1.0 / R, None, ALU.mult)
            m2 = tp.tile([D, 1], F32, tag="m2")
            nc.vector.tensor_scalar(m2[:], qt, 1.0 / R, None, ALU.mult)
            musq = tp.tile([D, 1], F32, tag="musq")
            nc.vector.tensor_tensor(musq[:], muH[:], muH[:], op=ALU.mult)
            # per-batch ga means: mb = bs4/N - 1   (h = H - 1)
            mb = tp.tile([D, 4], F32, tag="mb")
            nc.vector.tensor_scalar(mb[:], bs4, 1.0 / N, -1.0,
                                    ALU.mult, ALU.add)
            mu2 = tp.tile([D, 1], F32, tag="mu2")
            nc.vector.tensor_reduce(mu2[:], mb[:], axis=AXX, op=ALU.add)
            nc.vector.tensor_scalar(mu2[:], mu2[:], 0.25, None, ALU.mult)
            mbsq = tp.tile([D, 4], F32, tag="mbsq")
            nc.vector.tensor_tensor(mbsq[:], mb[:], mb[:], op=ALU.mult)
            q2 = tp.tile([D, 1], F32, tag="q2")
            nc.vector.tensor_reduce(q2[:], mbsq[:], axis=AXX, op=ALU.add)
            nc.vector.tensor_scalar(q2[:], q2[:], 0.25, None, ALU.mult)
            mu2sq = tp.tile([D, 1], F32, tag="mu2sq")
            nc.vector.tensor_tensor(mu2sq[:], mu2[:], mu2[:], op=ALU.mult)
            # both rsqrt's DVE-only (fast-inverse-sqrt seed + 2 Newton steps);
            # keeps ACT in the exp_and_others table set (no table swaps)
            v2 = tp.tile([D, 2], F32, tag="v2")
            nc.vector.scalar_tensor_tensor(
                v2[:, 0:1], m2[:], EPS, musq[:], op0=ALU.add,
                op1=ALU.subtract)
            nc.vector.scalar_tensor_tensor(
                v2[:, 1:2], q2[:], EPS, mu2sq[:], op0=ALU.add,
                op1=ALU.subtract)
            lnv = tp.tile([D, 2], F32, tag="lnv")
            nc.scalar.activation(lnv[:], v2[:], AF.Ln)
            s12 = tp.tile([D, 2], F32, tag="s12")
            nc.scalar.activation(s12[:], lnv[:], AF.Exp, scale=-0.5)
            a1 = tp.tile([D, 1], F32, tag="a1")
            nc.vector.tensor_tensor(a1[:], g1, s12[:, 0:1], op=ALU.mult)
            wps = wp.tile([D, D], F16, tag="wps")
            nc.vector.tensor_scalar(wps[:], wt[:], a1[:], None, ALU.mult)
            ra1 = tp.tile([D, 1], F32, tag="ra1")
            nc.vector.reciprocal(ra1[:], a1[:])
            tv = tp.tile([D, 1], F32, tag="tv")
            nc.vector.scalar_tensor_tensor(
                tv[:], ra1[:], be1, muH[:], op0=ALU.mult, op1=ALU.subtract)
            tvh = tp.tile([D, 1], F16, tag="tvh")
            nc.vector.tensor_copy(tvh[:], tv[:])
            a2 = tp.tile([D, 1], F32, tag="a2")
            nc.vector.tensor_tensor(a2[:], g2, s12[:, 1:2], op=ALU.mult)
            gv = tp.tile([D, 4], F32, tag="gv")
            nc.vector.scalar_tensor_tensor(
                gv[:], mb[:], mu2[:], a2[:].broadcast_to((D, 4)),
                op0=ALU.subtract, op1=ALU.mult)
            nc.vector.tensor_scalar(gv[:], gv[:], be2, None, ALU.add)
            gvh = tp.tile([D, 4], F16, tag="gvh")
            nc.vector.tensor_copy(gvh[:], gv[:])
            up = ps.tile([D, Q], F32, tag="x")
            nc.tensor.matmul(up[:, 0:1], wps[:], tvh[:], start=True, stop=True)
            nc.tensor.matmul(up[:, 1:5], wb[:], gvh[:], start=True, stop=True)
            usb = tp.tile([D, 5], F32, tag="usb")
            nc.vector.tensor_copy(usb[:], up[:, 0:5])
            u4 = tp.tile([D, 4], F32, tag="u4")
            nc.vector.scalar_tensor_tensor(
                u4[:], usb[:, 1:5], bv, usb[:, 0:1].broadcast_to((D, 4)),
                op0=ALU.add, op1=ALU.add)
            # select this core's batch column
            um = tp.tile([D, 4], F32, tag="um")
            nc.vector.tensor_tensor(um[:], u4[:], sel_t[:], op=ALU.mult)
            uc = tp.tile([D, 1], F32, tag="uc")
            nc.vector.tensor_reduce(uc[:], um[:], axis=AXX, op=ALU.add)
            u1c = tp.tile([D, 1], F32, tag="u1c")
            nc.vector.tensor_scalar(u1c[:], uc[:], 1.0, None, ALU.add)
            return wps, uc, u1c

        # ---- conv1 (+ trunk init) + sublayer-0 elementwise ----
        hacc = tp.tile([D, NCH], F32, tag="hacc")
        qacc = tp.tile([D, NCH], F32, tag="qacc")
        for c in range(NCH):
            cs = slice(c * Q, (c + 1) * Q)
            xf = ep.tile([6, Q], F16, tag="xf")
            nc.sync.dma_start(xf[:], XFh[:, cs])
            pt = ps.tile([D, Q], F32, tag="x")
            for q in range(Q // 512):
                nc.tensor.matmul(pt[:, q * 512:(q + 1) * 512], w1_t[:],
                                 xf[:, q * 512:(q + 1) * 512],
                                 start=True, stop=True)
            nc.vector.tensor_scalar(Xp[:, cs], pt[:], cb1_t[:, 0:1], None,
                                    ALU.add)
            ew_A(c, hacc, qacc)

        # ---- 30 sublayers ----
        for k in range(NSUB):
            sg = collect(hacc, qacc)
            wps, uc, u1c = chain(k, sg)
            hacc = tp.tile([D, NCH], F32, tag="hacc")
            qacc = tp.tile([D, NCH], F32, tag="qacc")
            for c in range(NCH):
                cs = slice(c * Q, (c + 1) * Q)
                pt = ps.tile([D, Q], F32, tag="x")
                if k % 2 == 0:
                    # interior: x_{k+1} = W'^T H  (+u via ACT bias / STT scalar)
                    for q in range(Q // 512):
                        qs = slice(c * Q + q * 512, c * Q + (q + 1) * 512)
                        nc.tensor.matmul(pt[:, q * 512:(q + 1) * 512], wps[:],
                                         Ht[:, qs], start=True, stop=True)
                    ew_B(pt, uc[:, 0:1], u1c[:, 0:1], c, hacc, qacc)
                else:
                    # block end: X'_new = W'^T H + X'_old + u
                    for q in range(Q // 512):
                        qs = slice(c * Q + q * 512, c * Q + (q + 1) * 512)
                        nc.tensor.matmul(pt[:, q * 512:(q + 1) * 512], wps[:],
                                         Ht[:, qs], start=True, stop=False)
                        nc.tensor.matmul(pt[:, q * 512:(q + 1) * 512], id_t[:],
                                         Xp[:, qs], start=False, stop=True)
                    nc.vector.tensor_scalar(Xp[:, cs], pt[:], uc[:, 0:1],
                                            None, ALU.add)
                    ew_A(c, hacc, qacc)

        # ---- conv2: BN(128) then W2 + b2 + selector term ----
        g2c, be2c, b2c = cv_t[:, 0:1], cv_t[:, 1:2], cv_t[:, 2:3]
        sgf = collect(hacc, qacc)
        tot = tp.tile([D, 1], F32, tag="tot")
        nc.vector.tensor_reduce(tot[:], sgf[:, 0:4], axis=AXX, op=ALU.add)
        muH = tp.tile([D, 1], F32, tag="muH")
        nc.vector.tensor_scalar(muH[:], tot[:], 1.0 / R, None, ALU.mult)
        m2 = tp.tile([D, 1], F32, tag="m2")
        nc.vector.tensor_scalar(m2[:], sgf[:, 4:5], 1.0 / R, None, ALU.mult)
        musq = tp.tile([D, 1], F32, tag="musq")
        nc.vector.tensor_tensor(musq[:], muH[:], muH[:], op=ALU.mult)
        vf = tp.tile([D, 1], F32, tag="vf")
        nc.vector.scalar_tensor_tensor(
            vf[:], m2[:], EPS, musq[:], op0=ALU.add, op1=ALU.subtract)
        lnf = tp.tile([D, 1], F32, tag="lnf")
        nc.scalar.activation(lnf[:], vf[:], AF.Ln)
        sf = tp.tile([D, 1], F32, tag="sf")
        nc.scalar.activation(sf[:], lnf[:], AF.Exp, scale=-0.5)
        af = tp.tile([D, 1], F32, tag="af")
        nc.vector.tensor_tensor(af[:], g2c, sf[:], op=ALU.mult)
        w2p = wp.tile([D, 120], F16, tag="w2p")
        nc.vector.tensor_scalar(w2p[:], w2_t[:], af[:], None, ALU.mult)
        raf = tp.tile([D, 1], F32, tag="raf")
        nc.vector.reciprocal(raf[:], af[:])
        tvf = tp.tile([D, 1], F32, tag="tvf")
        nc.vector.scalar_tensor_tensor(
            tvf[:], raf[:], be2c, muH[:], op0=ALU.mult, op1=ALU.subtract)
        tvfh = tp.tile([D, 1], F16, tag="tvfh")
        nc.vector.tensor_copy(tvfh[:], tvf[:])
        upf = ps.tile([D, Q], F32, tag="x")
        nc.tensor.matmul(upf[0:120, 0:1], w2p[:], tvfh[:], start=True,
                         stop=True)
        ufsb = tp.tile([D, 1], F32, tag="ufsb")
        nc.vector.tensor_tensor(ufsb[0:120, :], upf[0:120, 0:1],
                                b2c[0:120, :], op=ALU.add)
        for c in range(NCH):
            cs = slice(c * Q, (c + 1) * Q)
            xf3 = ep.tile([3, Q], F16, tag="xf")
            nc.sync.dma_start(xf3[:], XFh[3:6, cs])
            pt = ps.tile([120, Q], F32, tag="x")
            for q in range(Q // 512):
                qs = slice(c * Q + q * 512, c * Q + (q + 1) * 512)
                nc.tensor.matmul(pt[:, q * 512:(q + 1) * 512], w2p[:],
                                 Ht[:, qs], start=True, stop=False)
                nc.tensor.matmul(pt[:, q * 512:(q + 1) * 512], s_t[:],
                                 xf3[:, q * 512:(q + 1) * 512],
                                 start=False, stop=True)
            ot = op_.tile([120, Q], F32, tag="ot")
            nc.vector.tensor_scalar(ot[:], pt[:], ufsb[0:120, :], None,
                                    ALU.add)
            nc.sync.dma_start(OUT[:, cs], ot[:])

    nc.compile()
    return nc


def _prep(inputs):
    inp = np.asarray(inputs["inputs"], np.float32)          # [B, N, 6]
    rn_W = np.asarray(inputs["rn_W"], np.float32)           # [NB,2,256,128]
    rn_g = np.asarray(inputs["rn_gamma"], np.float32)       # [NB,2,256]
    rn_b = np.asarray(inputs["rn_beta"], np.float32)
    rn_bias = np.asarray(inputs["rn_b"], np.float32)        # [NB,2,128]
    XFa = np.ascontiguousarray(inp.reshape(R, 6).T).astype(np.float16)
    W1a = np.asarray(inputs["W1"], np.float32).astype(np.float16)
    WT = rn_W[:, :, :D, :].reshape(NSUB, D, D).astype(np.float16)
    WB = rn_W[:, :, D:, :].reshape(NSUB, D, D).astype(np.float16)
    PKa = np.zeros((D, NSUB * 8), np.float32)
    for kk in range(NSUB):
        l, j = kk // 2, kk % 2
        PKa[:, kk * 8 + 0] = rn_g[l, j, :D]
        PKa[:, kk * 8 + 1] = rn_b[l, j, :D]
        PKa[:, kk * 8 + 2] = rn_g[l, j, D:]
        PKa[:, kk * 8 + 3] = rn_b[l, j, D:]
        PKa[:, kk * 8 + 4] = rn_bias[l, j]
    CB1a = (np.asarray(inputs["b1"], np.float32) + 1.0).reshape(D, 1)
    W2a = np.asarray(inputs["W2"], np.float32).astype(np.float16)
    Sa = np.zeros((3, 120), np.float16)
    for f in range(120):
        Sa[f % 3, f] = 1.0
    CVa = np.zeros((D, 4), np.float32)
    CVa[:, 0] = np.asarray(inputs["g2"], np.float32)
    CVa[:, 1] = np.asarray(inputs["be2"], np.float32)
    CVa[:120, 2] = np.asarray(inputs["b2"], np.float32)
    IDa = np.eye(D, dtype=np.float16)
    common = {"W1h": W1a, "WTh": WT, "WBh": WB, "PK": PKa, "CB1": CB1a,
              "W2h": W2a, "Sh": Sa, "CV": CVa, "IDm": IDa}
    maps = []
    for c in range(NCORES):
        sel = np.zeros((D, 4), np.float32)
        sel[:, (c * RC) // N] = 1.0
        m = dict(common)
        m["XFh"] = np.ascontiguousarray(XFa[:, c * RC:(c + 1) * RC])
        m["SEL"] = sel
        maps.append(m)
    return maps


def _ref_numpy(inputs):
    """Exact fallback (unused for the spec'd all-ones mask)."""
    mask = np.asarray(inputs["mask"], np.float32)
    x = np.asarray(inputs["inputs"], np.float32)
    W1 = inputs["W1"]
    b1 = inputs["b1"]
    x = x @ W1 + b1

    def gbn(t, g, b):
        mu = t.mean((0, 1))
        v = ((t - mu) ** 2).mean((0, 1))
        return (t - mu) / np.sqrt(v + EPS) * g + b

    def gavg(t):
        return (t * mask).sum(1, keepdims=True) / mask.sum(1, keepdims=True)

    for l in range(NB):
        res = x
        for j in range(2):
            h = np.where(x > 0, x, np.expm1(np.minimum(x, 0)))
            ga = np.broadcast_to(gavg(h), h.shape)
            h = np.concatenate([h, ga], 2)
            h = gbn(h, inputs["rn_gamma"][l, j], inputs["rn_beta"][l, j])
            x = h @ inputs["rn_W"][l, j] + inputs["rn_b"][l, j]
        x = x + res
    h = np.where(x > 0, x, np.expm1(np.minimum(x, 0)))
    x = gbn(h, inputs["g2"], inputs["be2"]) @ inputs["W2"] + inputs["b2"]
    return (x + np.tile(np.asarray(inputs["inputs"])[:, :, -3:], (1, 1, 40))
            ).astype(np.float32)


def kernel(**inputs):
    mask = np.asarray(inputs["mask"], np.float32)
    if not (np.all(mask == 1.0) and np.asarray(inputs["inputs"]).shape ==
            (B, N, 6)):
        return _ref_numpy(inputs)
    if "nc" not in _CACHE:
        _CACHE["nc"] = _build()
    nc = _CACHE["nc"]
    maps = _prep(inputs)
    res = bass_utils.run_bass_kernel_spmd(
        nc, maps, core_ids=list(range(NCORES)))
    out = np.concatenate([res.results[c]["OUT"] for c in range(NCORES)],
                         axis=1)                     # [120, R]
    return np.ascontiguousarray(out.T).reshape(B, N, 120).astype(np.float32)
